# revision 27
# baseline (speedup 1.0000x reference)
"""Trainium2 Bass kernel for EfficientViT-style attention block.

Reference computation (per batch element b of 16):
    x: [256, 1024]  (C=256 channels, N=32*32 spatial)
    q = (sq*wq) @ x + bq        -> [128, N]  (8 heads x 16 key dims)
    k = (sk*wk) @ x + bk        -> [128, N]
    v = (sv*wv) @ x + bv        -> [256, N]  (8 heads x 32 v dims)
    per head: attn = softmax(q_h^T k_h, axis=-1); o_h = v_h @ attn^T
    out = (sp*wp) @ relu(concat o_h) + bp

Sharding: data-parallel over batch: 8 cores x 2 batch elements. No
collectives; full inputs sharded host-side, outputs concatenated.

Kernel strategy per core (all matmuls bf16 inputs, fp32 PSUM accumulate):
- scale factors folded into weights host-side; Q/K output channels
  pre-permuted into "padded head groups": group g holds heads 4g..4g+3 at
  32-partition stride so per-head score matmuls are tile_position-packable.
- scores computed transposed, ST[m, n] = k_h^T q_h, into PSUM; exp on
  ScalarE directly PSUM->SBUF (bf16). No max subtraction (|S| <= ~12).
- V bias rides through softmax (weights sum to 1) and is added at the end.
- AV: o_h += vT_h(m-tile)^T @ E(m-tile), 4 heads col-group-packed so the
  concat layout falls out in PSUM partitions.
- denominator: ones[128, 32] stationary against E -> denom replicated
  across each head's 32 partitions (reduction + broadcast in one matmul).
- normalize: o * recip(denom), + bias, relu on VectorE; final projection
  bf16 with f32 output + bias.
"""

import numpy as np
import ml_dtypes

B, C, H, W = 16, 256, 32, 32
N = H * W            # 1024
NH, KD, DV = 8, 16, 32
NB = 2               # batch elements per core
NCORES = 8
P = 128
NT = 512             # n-tile (psum bank)

BF16 = ml_dtypes.bfloat16

_CACHE = {}


def _build_nc():
    import concourse.tile as tile
    from concourse import bacc, mybir

    f32 = mybir.dt.float32
    bf16 = mybir.dt.bfloat16
    Alu = mybir.AluOpType
    Act = mybir.ActivationFunctionType

    # Bacc (not raw Bass): its finalize() runs generate_event_semaphores,
    # which splits multi-sem waits — TRN2 instructions take at most one.
    nc = bacc.Bacc()

    xb = nc.declare_dram_parameter("xb", [NB, C, N], bf16, isOutput=False)
    # all weights in one tensor: [tile, c, {wq|wk|wv|wp} x 256]
    wall = nc.declare_dram_parameter("wall", [2, P, 4 * 256], bf16,
                                     isOutput=False)
    # all per-partition bias vectors in one tensor: [partition, group, {q,k,v,p}]
    biases = nc.declare_dram_parameter("biases", [P, 2, 4], f32, isOutput=False)
    out = nc.declare_dram_parameter("out", [NB, C, N], f32, isOutput=True)

    with tile.TileContext(nc) as tc:
        with (
            tc.tile_pool(name="consts", bufs=1) as consts,
            tc.tile_pool(name="xp", bufs=2) as xp,
            tc.tile_pool(name="qk", bufs=2) as qk,
            tc.tile_pool(name="vtp", bufs=2) as vtp,
            tc.tile_pool(name="ep", bufs=3) as ep,
            tc.tile_pool(name="rp", bufs=2) as rp,
            tc.tile_pool(name="yp", bufs=4) as yp,
            tc.tile_pool(name="ps_s", bufs=1, space="PSUM") as ps_s,
            tc.tile_pool(name="ps_av", bufs=1, space="PSUM") as ps_av,
            tc.tile_pool(name="ps_den", bufs=1, space="PSUM") as ps_den,
            tc.tile_pool(name="ps_proj", bufs=2, space="PSUM") as ps_proj,
        ):
            # --- memsets + PE warmup first (no DMA deps): HAM ramps to
            # full clock while the input DMAs are in flight ---
            ones_sb = consts.tile([P, DV], bf16, tag="ones")
            nc.vector.memset(ones_sb[:], 1.0)
            warm_rhs = consts.tile([P, NT], bf16, tag="warm_rhs")
            nc.vector.memset(warm_rhs[:], 0.0)
            for wi in range(9):
                wps = ps_proj.tile([P, NT], f32, tag="proj",
                                   name=f"warm{wi}")
                nc.tensor.matmul(wps[0:DV, :], lhsT=ones_sb[:],
                                 rhs=warm_rhs[:], start=True, stop=True)

            # --- constants into SBUF: x for b=0 first (gates the first
            # projections), then weights, then biases ---
            x0_sb = xp.tile([P, 2, N], bf16, tag="x")
            for ct in range(2):
                nc.sync.dma_start(out=x0_sb[:, ct, :],
                                  in_=xb[0, ct * P:(ct + 1) * P, :])
            w_sb = consts.tile([P, 2, 4 * 256], bf16, tag="w")
            for ct in range(2):
                nc.gpsimd.dma_start(out=w_sb[:, ct, :], in_=wall[ct])
            bias_sb = consts.tile([P, 2, 4], f32, tag="bias")
            nc.sync.dma_start(out=bias_sb[:], in_=biases[:])
            # "touch" ops: bring the DVE/ACT vector clocks past the constant
            # DMAs so downstream TensorScalar instructions (1-wait-limited in
            # walrus codegen) only ever wait on the PE semaphore. The ACT
            # touch also pre-loads the exp table set.
            scratch = consts.tile([P, 2], f32, tag="scratch")
            nc.vector.tensor_copy(out=scratch[:, 0:1], in_=bias_sb[:, 0, 0:1])
            nc.scalar.activation(out=scratch[:, 1:2], in_=bias_sb[:, 0, 1:2],
                                 func=Act.Exp)

            def bias_ap(kind, g):
                i = {"q": 0, "k": 1, "v": 2, "p": 3}[kind]
                return bias_sb[:, g, i:i + 1]


            # ---------- per-b building blocks ----------
            qkv = {}    # b -> dict(q=, k=, vt=)
            r_tiles = {}  # b -> r_sb

            def qkv_piece(b, kind, g, nt_or_mt):
                """One projection piece: kind in {q, k, vt}."""
                x_sb = qkv[b]["x"]
                if kind in ("q", "k"):
                    woff = 0 if kind == "q" else 256
                    dst = qkv[b][kind]
                    nt = nt_or_mt
                    ps = ps_proj.tile([P, NT], f32, tag="proj",
                                      name=f"pp_{b}{kind}{g}{nt}")
                    for ct in range(2):
                        nc.tensor.matmul(
                            ps[:],
                            lhsT=w_sb[:, ct, woff + P * g:woff + P * (g + 1)],
                            rhs=x_sb[:, ct, nt * NT:(nt + 1) * NT],
                            start=(ct == 0), stop=(ct == 1))
                    nc.vector.tensor_scalar_add(
                        out=dst[:, g, nt * NT:(nt + 1) * NT],
                        in0=ps[:], scalar1=bias_ap(kind, g))
                else:
                    mt = nt_or_mt
                    ps = ps_proj.tile([P, NT], f32, tag="proj",
                                      name=f"pv_{b}{mt}")
                    for ct in range(2):
                        nc.tensor.matmul(
                            ps[:, 0:256],
                            lhsT=x_sb[:, ct, mt * P:(mt + 1) * P],
                            rhs=w_sb[:, ct, 512:768],
                            start=(ct == 0), stop=(ct == 1))
                    nc.vector.tensor_copy(out=qkv[b]["vt"][:, mt, :],
                                          in_=ps[:, 0:256])

            def emit_qkv_head(b):
                """Allocate b's tiles + the minimum pieces for its first
                scores: k(g0, both nt) and q(g0, nt0). Returns the deferred
                piece closures to spread into the pipeline."""
                if b == 0:
                    x_sb = x0_sb
                else:
                    x_sb = xp.tile([P, 2, N], bf16, tag="x", name=f"x{b}")
                    for ct in range(2):
                        nc.sync.dma_start(out=x_sb[:, ct, :],
                                          in_=xb[b, ct * P:(ct + 1) * P, :])
                qkv[b] = dict(
                    x=x_sb,
                    q=qk.tile([P, 2, N], bf16, tag="q", name=f"q{b}"),
                    k=qk.tile([P, 2, N], bf16, tag="k", name=f"k{b}"),
                    vt=vtp.tile([P, 8, 256], bf16, tag="vt", name=f"vt{b}"))
                r_tiles[b] = rp.tile([P, 2, N], bf16, tag="r", name=f"r{b}")
                for kind, g, i in (("k", 0, 0), ("k", 0, 1), ("q", 0, 0)):
                    qkv_piece(b, kind, g, i)
                rest = [("q", 0, 1), ("q", 1, 0), ("k", 1, 0), ("k", 1, 1),
                        ("q", 1, 1)]
                rest += [("vt", 0, mt) for mt in range(8)]
                return [lambda kind=kind, g=g, i=i: qkv_piece(b, kind, g, i)
                        for kind, g, i in rest]

            def avden_chunks(pend):
                """The pending iteration's AV + denominator matmuls as 16
                chunks of 4 MMs. Pair (av_p, den_{p+1}) zipped for col-strip
                concurrency; chunk order keeps at most one open accumulation
                group per psum bank."""
                b, g, nt = pend["key"]
                av, den, e_all = pend["av"], pend["den"], pend["e"]
                vt_sb = qkv[b]["vt"]
                chunks = []
                for p in range(4):
                    q_ = (p + 1) % 4
                    h = 4 * g + p
                    for mt0 in range(0, 8, 2):
                        def chunk(p=p, q_=q_, h=h, mt0=mt0):
                            for mt in (mt0, mt0 + 1):
                                nc.tensor.matmul(
                                    av[32 * p:32 * p + 32, :],
                                    lhsT=vt_sb[:, mt, 32 * h:32 * h + 32],
                                    rhs=e_all[:, mt, p * NT:(p + 1) * NT],
                                    start=(mt == 0), stop=(mt == 7),
                                    tile_position=(0, 32 * p))
                                nc.tensor.matmul(
                                    den[32 * q_:32 * q_ + 32, :],
                                    lhsT=ones_sb[:],
                                    rhs=e_all[:, mt, q_ * NT:(q_ + 1) * NT],
                                    start=(mt == 0), stop=(mt == 7),
                                    tile_position=(0, 32 * q_))
                        chunks.append(chunk)
                return chunks

            def emit_finalize(pend):
                """normalize + bias + relu for the pending iteration; if it
                closes a batch element, also emit the output projection."""
                b, g, nt = pend["key"]
                av, den = pend["av"], pend["den"]
                recip = rp.tile([P, NT], f32, tag="recip")
                nc.vector.reciprocal_approx_fast(out=recip[:], in_=den[:])
                tmp = rp.tile([P, NT], f32, tag="tmp")
                nc.vector.scalar_tensor_tensor(
                    out=tmp[:], in0=av[:], scalar=1.0, in1=recip[:],
                    op0=Alu.bypass, op1=Alu.mult)
                nc.vector.tensor_scalar(
                    out=r_tiles[b][:, g, nt * NT:(nt + 1) * NT],
                    in0=tmp[:], scalar1=bias_ap("v", g),
                    scalar2=0.0, op0=Alu.add, op1=Alu.max)
                if (g, nt) == (1, 1):
                    r_sb = r_tiles[b]
                    for ct in range(2):
                        y_sb = yp.tile([P, N], f32, tag="y")
                        for nt2 in range(2):
                            ps = ps_proj.tile([P, NT], f32, tag="proj")
                            for gg in range(2):
                                nc.tensor.matmul(
                                    ps[:],
                                    lhsT=w_sb[:, gg, 768 + ct * P:768 + (ct + 1) * P],
                                    rhs=r_sb[:, gg, nt2 * NT:(nt2 + 1) * NT],
                                    start=(gg == 0), stop=(gg == 1))
                            nc.vector.tensor_scalar_add(
                                out=y_sb[:, nt2 * NT:(nt2 + 1) * NT],
                                in0=ps[:], scalar1=bias_ap("p", ct))
                        nc.sync.dma_start(
                            out=out[b, ct * P:(ct + 1) * P, :], in_=y_sb[:])

            # ---------- software-pipelined main loop ----------
            # iteration i: scores+exp for (b,g,nt), interleaved with the
            # PREVIOUS iteration's AV/denominator chunks (keeps the PE dense
            # so HAM stays at full clock), then the previous normalize.
            pending = None
            qkv_queue = []
            for b in range(NB):
                for g in range(2):
                    for nt in range(2):
                        if b == 0 and (g, nt) == (0, 0):
                            qkv_queue.extend(emit_qkv_head(0))
                        if (b, g, nt) == (0, 1, 0) and NB > 1:
                            qkv_queue.extend(emit_qkv_head(1))
                        q_sb, k_sb = qkv[b]["q"], qkv[b]["k"]
                        av = ps_av.tile([P, NT], f32, tag="av")
                        den = ps_den.tile([P, NT], f32, tag="den")
                        e_all = ep.tile([P, 8, 4 * NT], bf16, tag="e")
                        chunks = avden_chunks(pending) if pending else []
                        ci = 0
                        for mt in range(8):
                            # 4-way row-group-packed score matmuls (one span)
                            sts = [ps_s.tile([P, 2 * NT], f32, tag=t,
                                             name=f"s_{b}{g}{nt}{mt}{t}")
                                   for t in ("sa", "sb")]
                            for j in range(4):
                                row = 32 * j
                                nc.tensor.matmul(
                                    sts[j // 2][:, (j % 2) * NT:
                                                (j % 2 + 1) * NT],
                                    lhsT=k_sb[row:row + KD, g,
                                              mt * P:(mt + 1) * P],
                                    rhs=q_sb[row:row + KD, g,
                                             nt * NT:(nt + 1) * NT],
                                    start=True, stop=True,
                                    tile_position=(row, 0))
                            for half in range(2):
                                nc.scalar.activation(
                                    out=e_all[:, mt,
                                              half * 2 * NT:(half + 1) * 2 * NT],
                                    in_=sts[half][:], func=Act.Exp)
                                hs = 2 * mt + half
                                while (hs >= 2 and ci < len(chunks)
                                       and ci < 2 * (hs - 1)):
                                    chunks[ci]()
                                    ci += 1
                                if not pending and hs >= 2:
                                    nc.tensor.matmul(
                                        av[0:DV, :], lhsT=ones_sb[:],
                                        rhs=warm_rhs[:],
                                        start=True, stop=True)
                                drain_all = (b, g, nt) == (0, 0, 0)
                                if (drain_all or hs % 2 == 1) and qkv_queue:
                                    qkv_queue.pop(0)()
                        while ci < len(chunks):
                            chunks[ci]()
                            ci += 1
                        if pending:
                            emit_finalize(pending)
                        pending = dict(key=(b, g, nt), av=av, den=den, e=e_all)
            # ---- drain the last iteration: after the final exp, sa/sb
            # and proj banks are free, so every AV/den accumulation group
            # gets its own psum bank -> 4-way col-strip concurrency ----
            b, g, nt = pending["key"]
            e_all = pending["e"]
            vt_sb = qkv[b]["vt"]
            hostA = ps_proj.tile([P, NT], f32, tag="proj", name="drA")
            hostB = ps_proj.tile([P, NT], f32, tag="proj", name="drB")
            hostSA = ps_s.tile([P, 2 * NT], f32, tag="sa", name="drSA")
            hostSB = ps_s.tile([P, 2 * NT], f32, tag="sb", name="drSB")
            av_hosts = [(hostSA, 0), (hostSB, 0),
                        (pending["av"], 0), (pending["den"], 0)]
            den_hosts = [(hostA, 0), (hostB, 0),
                         (hostSA, NT), (hostSB, NT)]
            # den hosts hold one 32-row group each; fill the rest so the
            # full-tile recip (custom-DVE needs partition base 0) reads
            # defined data (recip(1.0) in unused rows is harmless)
            for host, fo in den_hosts:
                nc.vector.memset(host[:, fo:fo + NT], 1.0)
            for mt in range(8):
                for p in range(4):
                    host, fo = av_hosts[p]
                    h = 4 * g + p
                    nc.tensor.matmul(
                        host[32 * p:32 * p + 32, fo:fo + NT],
                        lhsT=vt_sb[:, mt, 32 * h:32 * h + 32],
                        rhs=e_all[:, mt, p * NT:(p + 1) * NT],
                        start=(mt == 0), stop=(mt == 7),
                        tile_position=(0, 32 * p))
                for q_ in range(4):
                    host, fo = den_hosts[q_]
                    nc.tensor.matmul(
                        host[32 * q_:32 * q_ + 32, fo:fo + NT],
                        lhsT=ones_sb[:],
                        rhs=e_all[:, mt, q_ * NT:(q_ + 1) * NT],
                        start=(mt == 0), stop=(mt == 7),
                        tile_position=(0, 32 * q_))
            # finalize: full-host recips (custom-DVE needs partition base 0),
            # per-head normalize from the scattered hosts
            tmp = rp.tile([P, NT], f32, tag="tmp")
            recips = []
            for i, (host, fo) in enumerate(den_hosts):
                rec = rp.tile([P, NT], f32, tag=f"drrec{i}",
                              name=f"drrec{i}")
                nc.vector.reciprocal_approx_fast(
                    out=rec[:], in_=host[:, fo:fo + NT])
                recips.append(rec)
            for p in range(4):
                avh, afo = av_hosts[p]
                nc.vector.scalar_tensor_tensor(
                    out=tmp[32 * p:32 * p + 32, :],
                    in0=avh[32 * p:32 * p + 32, afo:afo + NT], scalar=1.0,
                    in1=recips[p][32 * p:32 * p + 32, :],
                    op0=Alu.bypass, op1=Alu.mult)
            nc.vector.tensor_scalar(
                out=r_tiles[b][:, g, nt * NT:(nt + 1) * NT],
                in0=tmp[:], scalar1=bias_ap("v", g),
                scalar2=0.0, op0=Alu.add, op1=Alu.max)
            for ct in range(2):
                y_sb = yp.tile([P, N], f32, tag="y", name=f"ydr{ct}")
                for nt2 in range(2):
                    ps = ps_proj.tile([P, NT], f32, tag="proj",
                                      name=f"ydrp{ct}{nt2}")
                    for gg in range(2):
                        nc.tensor.matmul(
                            ps[:],
                            lhsT=w_sb[:, gg, 768 + ct * P:768 + (ct + 1) * P],
                            rhs=r_tiles[b][:, gg, nt2 * NT:(nt2 + 1) * NT],
                            start=(gg == 0), stop=(gg == 1))
                    nc.vector.tensor_scalar_add(
                        out=y_sb[:, nt2 * NT:(nt2 + 1) * NT],
                        in0=ps[:], scalar1=bias_ap("p", ct))
                nc.sync.dma_start(
                    out=out[b, ct * P:(ct + 1) * P, :], in_=y_sb[:])

    if not nc.is_finalized():
        nc.finalize()
    return nc


def _prep_consts(wq, sq, bq, wk, sk, bk, wv, sv, bv, wp, sp, bp):
    """Host-side weight prep. Returns dict of per-core-identical arrays."""
    wq_s = (sq[:, None] * wq).astype(np.float32)
    wk_s = (sk[:, None] * wk).astype(np.float32)
    wv_s = (sv[:, None] * wv).astype(np.float32)
    wp_s = (sp[:, None] * wp).astype(np.float32)

    def pad_qk(w_s, bias):
        wT_pad = np.zeros((256, 256), np.float32)   # [c, gcol]
        b_pad = np.zeros(256, np.float32)
        for g in range(2):
            for j in range(4):
                h = 4 * g + j
                col = 128 * g + 32 * j
                wT_pad[:, col:col + KD] = w_s[KD * h:KD * (h + 1), :].T
                b_pad[col:col + KD] = bias[KD * h:KD * (h + 1)]
        return (wT_pad.reshape(2, P, 256).astype(BF16),
                b_pad.reshape(2, P, 1).astype(np.float32))

    wqT, bqp = pad_qk(wq_s, bq)
    wkT, bkp = pad_qk(wk_s, bk)
    wvT = wv_s.T.copy().reshape(2, P, 256).astype(BF16)   # [c, dh]
    wpT = wp_s.T.copy().reshape(2, P, 256).astype(BF16)   # [dh, c]
    wall = np.concatenate([wqT, wkT, wvT, wpT], axis=2)   # [2, 128, 1024]
    bvp = bv.reshape(2, P).astype(np.float32)
    bpp = bp.reshape(2, P).astype(np.float32)
    # combined bias tensor: [partition, group, {q,k,v,p}]
    biases = np.zeros((P, 2, 4), np.float32)
    for g in range(2):
        biases[:, g, 0] = bqp[g, :, 0]
        biases[:, g, 1] = bkp[g, :, 0]
        biases[:, g, 2] = bvp[g]
        biases[:, g, 3] = bpp[g]
    return dict(wall=wall, biases=biases)


def make_in_maps(inputs):
    x = np.ascontiguousarray(inputs["x"]).reshape(B, C, N).astype(BF16)
    consts = _prep_consts(*[np.asarray(inputs[k], np.float32) for k in
                            ["wq", "sq", "bq", "wk", "sk", "bk",
                             "wv", "sv", "bv", "wp", "sp", "bp"]])
    in_maps = []
    for core in range(NCORES):
        m = dict(consts)
        m["xb"] = np.ascontiguousarray(x[NB * core:NB * (core + 1)])
        in_maps.append(m)
    return in_maps


def gather_out(results):
    parts = [np.asarray(results[i]["out"], np.float32) for i in range(NCORES)]
    return np.concatenate(parts, axis=0).reshape(B, C, H, W)


def get_nc():
    if "nc" not in _CACHE:
        _CACHE["nc"] = _build_nc()
    return _CACHE["nc"]


def kernel(**inputs):
    import os
    os.environ.setdefault("BASS_NEVER_TRACE", "1")
    from concourse.bass_utils import run_bass_kernel_spmd
    nc = get_nc()
    in_maps = make_in_maps(inputs)
    res = run_bass_kernel_spmd(nc, in_maps, core_ids=list(range(NCORES)),
                               trace=False)
    return gather_out(res.results)


if __name__ == "__main__":
    nc = _build_nc()
    print("built ok; instructions:",
          sum(len(bb.instructions) for f in nc.m.functions
              for bb in getattr(f, "basic_blocks", [])) if hasattr(
                  nc.m.functions[0], "basic_blocks") else "n/a")


# revision 28
# speedup vs baseline: 1.0213x; 1.0213x over previous
"""Trainium2 Bass kernel for EfficientViT-style attention block.

Reference computation (per batch element b of 16):
    x: [256, 1024]  (C=256 channels, N=32*32 spatial)
    q = (sq*wq) @ x + bq        -> [128, N]  (8 heads x 16 key dims)
    k = (sk*wk) @ x + bk        -> [128, N]
    v = (sv*wv) @ x + bv        -> [256, N]  (8 heads x 32 v dims)
    per head: attn = softmax(q_h^T k_h, axis=-1); o_h = v_h @ attn^T
    out = (sp*wp) @ relu(concat o_h) + bp

Sharding: data-parallel over batch: 8 cores x 2 batch elements. No
collectives; full inputs sharded host-side, outputs concatenated.

Kernel strategy per core (all matmuls bf16 inputs, fp32 PSUM accumulate):
- scale factors folded into weights host-side; Q/K output channels
  pre-permuted into "padded head groups": group g holds heads 4g..4g+3 at
  32-partition stride so per-head score matmuls are tile_position-packable.
- scores computed transposed, ST[m, n] = k_h^T q_h, into PSUM; exp on
  ScalarE directly PSUM->SBUF (bf16). No max subtraction (|S| <= ~12).
- V bias rides through softmax (weights sum to 1) and is added at the end.
- AV: o_h += vT_h(m-tile)^T @ E(m-tile), 4 heads col-group-packed so the
  concat layout falls out in PSUM partitions.
- denominator: ones[128, 32] stationary against E -> denom replicated
  across each head's 32 partitions (reduction + broadcast in one matmul).
- normalize: o * recip(denom), + bias, relu on VectorE; final projection
  bf16 with f32 output + bias.
"""

import numpy as np
import ml_dtypes

B, C, H, W = 16, 256, 32, 32
N = H * W            # 1024
NH, KD, DV = 8, 16, 32
NB = 2               # batch elements per core
NCORES = 8
P = 128
NT = 512             # n-tile (psum bank)

BF16 = ml_dtypes.bfloat16

_CACHE = {}


def _build_nc():
    import concourse.tile as tile
    from concourse import bacc, mybir

    f32 = mybir.dt.float32
    bf16 = mybir.dt.bfloat16
    Alu = mybir.AluOpType
    Act = mybir.ActivationFunctionType

    # Bacc (not raw Bass): its finalize() runs generate_event_semaphores,
    # which splits multi-sem waits — TRN2 instructions take at most one.
    nc = bacc.Bacc()

    xb = nc.declare_dram_parameter("xb", [NB, C, N], bf16, isOutput=False)
    # all weights in one tensor: [tile, c, {wq|wk|wv|wp} x 256]
    wall = nc.declare_dram_parameter("wall", [2, P, 4 * 256], bf16,
                                     isOutput=False)
    # all per-partition bias vectors in one tensor: [partition, group, {q,k,v,p}]
    biases = nc.declare_dram_parameter("biases", [P, 2, 4], f32, isOutput=False)
    out = nc.declare_dram_parameter("out", [NB, C, N], f32, isOutput=True)

    with tile.TileContext(nc) as tc:
        with (
            tc.tile_pool(name="consts", bufs=1) as consts,
            tc.tile_pool(name="xp", bufs=2) as xp,
            tc.tile_pool(name="qk", bufs=2) as qk,
            tc.tile_pool(name="vtp", bufs=2) as vtp,
            tc.tile_pool(name="ep", bufs=3) as ep,
            tc.tile_pool(name="rp", bufs=2) as rp,
            tc.tile_pool(name="yp", bufs=4) as yp,
            tc.tile_pool(name="ps_s", bufs=1, space="PSUM") as ps_s,
            tc.tile_pool(name="ps_av", bufs=1, space="PSUM") as ps_av,
            tc.tile_pool(name="ps_den", bufs=1, space="PSUM") as ps_den,
            tc.tile_pool(name="ps_proj", bufs=2, space="PSUM") as ps_proj,
        ):
            # --- memsets + PE warmup first (no DMA deps): HAM ramps to
            # full clock while the input DMAs are in flight ---
            ones_sb = consts.tile([P, DV], bf16, tag="ones")
            nc.vector.memset(ones_sb[:], 1.0)
            warm_rhs = consts.tile([P, NT], bf16, tag="warm_rhs")
            nc.vector.memset(warm_rhs[:], 0.0)
            for wi in range(9):
                wps = ps_proj.tile([P, NT], f32, tag="proj",
                                   name=f"warm{wi}")
                nc.tensor.matmul(wps[0:DV, :], lhsT=ones_sb[:],
                                 rhs=warm_rhs[:], start=True, stop=True)

            # --- constants into SBUF: x for b=0 first (gates the first
            # projections), then weights, then biases ---
            x0_sb = xp.tile([P, 2, N], bf16, tag="x")
            for ct in range(2):
                nc.sync.dma_start(out=x0_sb[:, ct, :],
                                  in_=xb[0, ct * P:(ct + 1) * P, :])
            w_sb = consts.tile([P, 2, 4 * 256], bf16, tag="w")
            for ct in range(2):
                nc.gpsimd.dma_start(out=w_sb[:, ct, :], in_=wall[ct])
            bias_sb = consts.tile([P, 2, 4], f32, tag="bias")
            nc.sync.dma_start(out=bias_sb[:], in_=biases[:])
            # "touch" ops: bring the DVE/ACT vector clocks past the constant
            # DMAs so downstream TensorScalar instructions (1-wait-limited in
            # walrus codegen) only ever wait on the PE semaphore. The ACT
            # touch also pre-loads the exp table set.
            scratch = consts.tile([P, 2], f32, tag="scratch")
            nc.vector.tensor_copy(out=scratch[:, 0:1], in_=bias_sb[:, 0, 0:1])
            nc.scalar.activation(out=scratch[:, 1:2], in_=bias_sb[:, 0, 1:2],
                                 func=Act.Exp)

            def bias_ap(kind, g):
                i = {"q": 0, "k": 1, "v": 2, "p": 3}[kind]
                return bias_sb[:, g, i:i + 1]


            # ---------- per-b building blocks ----------
            qkv = {}    # b -> dict(q=, k=, vt=)
            r_tiles = {}  # b -> r_sb

            def qkv_piece(b, kind, g, nt_or_mt):
                """One projection piece: kind in {q, k, vt}."""
                x_sb = qkv[b]["x"]
                if kind in ("q", "k"):
                    woff = 0 if kind == "q" else 256
                    dst = qkv[b][kind]
                    nt = nt_or_mt
                    ps = ps_proj.tile([P, NT], f32, tag="proj",
                                      name=f"pp_{b}{kind}{g}{nt}")
                    for ct in range(2):
                        nc.tensor.matmul(
                            ps[:],
                            lhsT=w_sb[:, ct, woff + P * g:woff + P * (g + 1)],
                            rhs=x_sb[:, ct, nt * NT:(nt + 1) * NT],
                            start=(ct == 0), stop=(ct == 1))
                    nc.vector.tensor_scalar_add(
                        out=dst[:, g, nt * NT:(nt + 1) * NT],
                        in0=ps[:], scalar1=bias_ap(kind, g))
                else:
                    mt = nt_or_mt
                    ps = ps_proj.tile([P, NT], f32, tag="proj",
                                      name=f"pv_{b}{mt}")
                    for ct in range(2):
                        nc.tensor.matmul(
                            ps[:, 0:256],
                            lhsT=x_sb[:, ct, mt * P:(mt + 1) * P],
                            rhs=w_sb[:, ct, 512:768],
                            start=(ct == 0), stop=(ct == 1))
                    nc.vector.tensor_copy(out=qkv[b]["vt"][:, mt, :],
                                          in_=ps[:, 0:256])

            def emit_qkv_head(b):
                """Allocate b's tiles + the minimum pieces for its first
                scores: k(g0, both nt) and q(g0, nt0). Returns the deferred
                piece closures to spread into the pipeline."""
                if b == 0:
                    x_sb = x0_sb
                else:
                    x_sb = xp.tile([P, 2, N], bf16, tag="x", name=f"x{b}")
                    for ct in range(2):
                        nc.sync.dma_start(out=x_sb[:, ct, :],
                                          in_=xb[b, ct * P:(ct + 1) * P, :])
                qkv[b] = dict(
                    x=x_sb,
                    q=qk.tile([P, 2, N], bf16, tag="q", name=f"q{b}"),
                    k=qk.tile([P, 2, N], bf16, tag="k", name=f"k{b}"),
                    vt=vtp.tile([P, 8, 256], bf16, tag="vt", name=f"vt{b}"))
                r_tiles[b] = rp.tile([P, 2, N], bf16, tag="r", name=f"r{b}")
                for kind, g, i in (("k", 0, 0), ("k", 0, 1), ("q", 0, 0)):
                    qkv_piece(b, kind, g, i)
                rest = [("q", 0, 1), ("q", 1, 0), ("k", 1, 0), ("k", 1, 1),
                        ("q", 1, 1)]
                rest += [("vt", 0, mt) for mt in range(8)]
                return [lambda kind=kind, g=g, i=i: qkv_piece(b, kind, g, i)
                        for kind, g, i in rest]

            def avden_chunks(pend):
                """The pending iteration's AV + denominator matmuls as 16
                chunks of 4 MMs. Pair (av_p, den_{p+1}) zipped for col-strip
                concurrency; chunk order keeps at most one open accumulation
                group per psum bank."""
                b, g, nt = pend["key"]
                av, den, e_all = pend["av"], pend["den"], pend["e"]
                vt_sb = qkv[b]["vt"]
                chunks = []
                for p in range(4):
                    q_ = (p + 1) % 4
                    h = 4 * g + p
                    for mt0 in range(0, 8, 2):
                        def chunk(p=p, q_=q_, h=h, mt0=mt0):
                            for mt in (mt0, mt0 + 1):
                                nc.tensor.matmul(
                                    av[32 * p:32 * p + 32, :],
                                    lhsT=vt_sb[:, mt, 32 * h:32 * h + 32],
                                    rhs=e_all[:, mt, p * NT:(p + 1) * NT],
                                    start=(mt == 0), stop=(mt == 7),
                                    tile_position=(0, 32 * p))
                                nc.tensor.matmul(
                                    den[32 * q_:32 * q_ + 32, :],
                                    lhsT=ones_sb[:],
                                    rhs=e_all[:, mt, q_ * NT:(q_ + 1) * NT],
                                    start=(mt == 0), stop=(mt == 7),
                                    tile_position=(0, 32 * q_))
                        chunks.append(chunk)
                return chunks

            def emit_finalize(pend):
                """normalize + bias + relu for the pending iteration; if it
                closes a batch element, also emit the output projection."""
                b, g, nt = pend["key"]
                av, den = pend["av"], pend["den"]
                recip = rp.tile([P, NT], f32, tag="recip")
                nc.vector.reciprocal_approx_fast(out=recip[:], in_=den[:])
                tmp = rp.tile([P, NT], f32, tag="tmp")
                nc.vector.scalar_tensor_tensor(
                    out=tmp[:], in0=av[:], scalar=1.0, in1=recip[:],
                    op0=Alu.bypass, op1=Alu.mult)
                nc.vector.tensor_scalar(
                    out=r_tiles[b][:, g, nt * NT:(nt + 1) * NT],
                    in0=tmp[:], scalar1=bias_ap("v", g),
                    scalar2=0.0, op0=Alu.add, op1=Alu.max)
                if (g, nt) == (1, 1):
                    r_sb = r_tiles[b]
                    for ct in range(2):
                        y_sb = yp.tile([P, N], f32, tag="y")
                        for nt2 in range(2):
                            ps = ps_proj.tile([P, NT], f32, tag="proj")
                            for gg in range(2):
                                nc.tensor.matmul(
                                    ps[:],
                                    lhsT=w_sb[:, gg, 768 + ct * P:768 + (ct + 1) * P],
                                    rhs=r_sb[:, gg, nt2 * NT:(nt2 + 1) * NT],
                                    start=(gg == 0), stop=(gg == 1))
                            nc.vector.tensor_scalar_add(
                                out=y_sb[:, nt2 * NT:(nt2 + 1) * NT],
                                in0=ps[:], scalar1=bias_ap("p", ct))
                        nc.sync.dma_start(
                            out=out[b, ct * P:(ct + 1) * P, :], in_=y_sb[:])

            # ---------- software-pipelined main loop ----------
            # iteration i: scores+exp for (b,g,nt), interleaved with the
            # PREVIOUS iteration's AV/denominator chunks (keeps the PE dense
            # so HAM stays at full clock), then the previous normalize.
            pending = None
            qkv_queue = []
            for b in range(NB):
                for g in range(2):
                    for nt in range(2):
                        if b == 0 and (g, nt) == (0, 0):
                            qkv_queue.extend(emit_qkv_head(0))
                        if (b, g, nt) == (0, 1, 0) and NB > 1:
                            qkv_queue.extend(emit_qkv_head(1))
                        q_sb, k_sb = qkv[b]["q"], qkv[b]["k"]
                        av = ps_av.tile([P, NT], f32, tag="av")
                        den = ps_den.tile([P, NT], f32, tag="den")
                        e_all = ep.tile([P, 8, 4 * NT], bf16, tag="e")
                        chunks = avden_chunks(pending) if pending else []
                        ci = 0
                        for mt in range(8):
                            # 4-way row-group-packed score matmuls (one span)
                            sts = [ps_s.tile([P, 2 * NT], f32, tag=t,
                                             name=f"s_{b}{g}{nt}{mt}{t}")
                                   for t in ("sa", "sb")]
                            for j in range(4):
                                row = 32 * j
                                nc.tensor.matmul(
                                    sts[j // 2][:, (j % 2) * NT:
                                                (j % 2 + 1) * NT],
                                    lhsT=k_sb[row:row + KD, g,
                                              mt * P:(mt + 1) * P],
                                    rhs=q_sb[row:row + KD, g,
                                             nt * NT:(nt + 1) * NT],
                                    start=True, stop=True,
                                    tile_position=(row, 0))
                            for half in range(2):
                                nc.scalar.activation(
                                    out=e_all[:, mt,
                                              half * 2 * NT:(half + 1) * 2 * NT],
                                    in_=sts[half][:], func=Act.Exp)
                                hs = 2 * mt + half
                                while (hs >= 2 and ci < len(chunks)
                                       and ci < 2 * (hs - 1)):
                                    chunks[ci]()
                                    ci += 1
                                if not pending and hs >= 2:
                                    nc.tensor.matmul(
                                        av[0:DV, :], lhsT=ones_sb[:],
                                        rhs=warm_rhs[:],
                                        start=True, stop=True)
                                drain_all = (b, g, nt) == (0, 0, 0)
                                if (drain_all or hs % 2 == 1) and qkv_queue:
                                    qkv_queue.pop(0)()
                        while ci < len(chunks):
                            chunks[ci]()
                            ci += 1
                        if pending:
                            emit_finalize(pending)
                        pending = dict(key=(b, g, nt), av=av, den=den, e=e_all)
            # drain the last iteration
            for chunk in avden_chunks(pending):
                chunk()
            emit_finalize(pending)

    if not nc.is_finalized():
        nc.finalize()
    return nc


def _prep_consts(wq, sq, bq, wk, sk, bk, wv, sv, bv, wp, sp, bp):
    """Host-side weight prep. Returns dict of per-core-identical arrays."""
    wq_s = (sq[:, None] * wq).astype(np.float32)
    wk_s = (sk[:, None] * wk).astype(np.float32)
    wv_s = (sv[:, None] * wv).astype(np.float32)
    wp_s = (sp[:, None] * wp).astype(np.float32)

    def pad_qk(w_s, bias):
        wT_pad = np.zeros((256, 256), np.float32)   # [c, gcol]
        b_pad = np.zeros(256, np.float32)
        for g in range(2):
            for j in range(4):
                h = 4 * g + j
                col = 128 * g + 32 * j
                wT_pad[:, col:col + KD] = w_s[KD * h:KD * (h + 1), :].T
                b_pad[col:col + KD] = bias[KD * h:KD * (h + 1)]
        return (wT_pad.reshape(2, P, 256).astype(BF16),
                b_pad.reshape(2, P, 1).astype(np.float32))

    wqT, bqp = pad_qk(wq_s, bq)
    wkT, bkp = pad_qk(wk_s, bk)
    wvT = wv_s.T.copy().reshape(2, P, 256).astype(BF16)   # [c, dh]
    wpT = wp_s.T.copy().reshape(2, P, 256).astype(BF16)   # [dh, c]
    wall = np.concatenate([wqT, wkT, wvT, wpT], axis=2)   # [2, 128, 1024]
    bvp = bv.reshape(2, P).astype(np.float32)
    bpp = bp.reshape(2, P).astype(np.float32)
    # combined bias tensor: [partition, group, {q,k,v,p}]
    biases = np.zeros((P, 2, 4), np.float32)
    for g in range(2):
        biases[:, g, 0] = bqp[g, :, 0]
        biases[:, g, 1] = bkp[g, :, 0]
        biases[:, g, 2] = bvp[g]
        biases[:, g, 3] = bpp[g]
    return dict(wall=wall, biases=biases)


def make_in_maps(inputs):
    x = np.ascontiguousarray(inputs["x"]).reshape(B, C, N).astype(BF16)
    consts = _prep_consts(*[np.asarray(inputs[k], np.float32) for k in
                            ["wq", "sq", "bq", "wk", "sk", "bk",
                             "wv", "sv", "bv", "wp", "sp", "bp"]])
    in_maps = []
    for core in range(NCORES):
        m = dict(consts)
        m["xb"] = np.ascontiguousarray(x[NB * core:NB * (core + 1)])
        in_maps.append(m)
    return in_maps


def gather_out(results):
    parts = [np.asarray(results[i]["out"], np.float32) for i in range(NCORES)]
    return np.concatenate(parts, axis=0).reshape(B, C, H, W)


def get_nc():
    if "nc" not in _CACHE:
        _CACHE["nc"] = _build_nc()
    return _CACHE["nc"]


def kernel(**inputs):
    import os
    os.environ.setdefault("BASS_NEVER_TRACE", "1")
    from concourse.bass_utils import run_bass_kernel_spmd
    nc = get_nc()
    in_maps = make_in_maps(inputs)
    res = run_bass_kernel_spmd(nc, in_maps, core_ids=list(range(NCORES)),
                               trace=False)
    return gather_out(res.results)


if __name__ == "__main__":
    nc = _build_nc()
    print("built ok; instructions:",
          sum(len(bb.instructions) for f in nc.m.functions
              for bb in getattr(f, "basic_blocks", [])) if hasattr(
                  nc.m.functions[0], "basic_blocks") else "n/a")
